# revision 24
# baseline (speedup 1.0000x reference)
"""Trainium2 Bass kernel for causal multi-head attention (B=2, T=2048, D=1024, H=16).

Sharding: 8 cores = 2 batches x 4 head-groups. Each core computes 4 heads
(as 2 head-pairs packed into 128 partitions) of one batch, plus its row-shard
of the output projection; the host sums the 4 partial outputs per batch.

Compute is bf16 with fp32 PSUM accumulation. Softmax uses no max-subtraction
(scores ~ N(0,1), exp is safe in fp32) and gets the denominator for free via
an all-ones column appended to V.
"""

import numpy as np
import ml_dtypes
from contextlib import ExitStack

import concourse.bass as bass
import concourse.mybir as mybir
import concourse.tile as tile
from concourse import bacc
from concourse.bass_utils import run_bass_kernel_spmd

BF16 = mybir.dt.bfloat16
F32 = mybir.dt.float32
AF = mybir.ActivationFunctionType
bf16 = ml_dtypes.bfloat16

B, T, D, H, DH = 2, 2048, 1024, 16, 64
NCORES = 8
QTILE = 512          # q columns per score tile
NQT = T // QTILE     # 4
TCH = T // 128       # 16 t-chunks / k-blocks

_CACHE = {}


def _build():
    nc = bacc.Bacc(
        "TRN2", target_bir_lowering=False, debug=False, num_devices=NCORES
    )
    xt_d = nc.dram_tensor("xt", [256, T], BF16, kind="ExternalInput").ap()
    wq_d = nc.dram_tensor("wq", [2, 128, 128], BF16, kind="ExternalInput").ap()
    wk_d = nc.dram_tensor("wk", [2, 128, 128], BF16, kind="ExternalInput").ap()
    wv_d = nc.dram_tensor("wv", [2, 128, 128], BF16, kind="ExternalInput").ap()
    bq_d = nc.dram_tensor("bq", [2, 128, 1], F32, kind="ExternalInput").ap()
    bk_d = nc.dram_tensor("bk", [2, 128, 1], F32, kind="ExternalInput").ap()
    wo_d = nc.dram_tensor("wo", [2, 128, D], BF16, kind="ExternalInput").ap()
    msk_d = nc.dram_tensor("msk", [128, 128], BF16, kind="ExternalInput").ap()
    y_d = nc.dram_tensor("y", [T, D], BF16, kind="ExternalOutput").ap()

    with tile.TileContext(nc) as tc, ExitStack() as ctx:
        const = ctx.enter_context(tc.tile_pool(name="const", bufs=1))
        pers = ctx.enter_context(tc.tile_pool(name="pers", bufs=1))
        pex = ctx.enter_context(tc.tile_pool(name="pex", bufs=2))
        patt = ctx.enter_context(tc.tile_pool(name="patt", bufs=4))
        pysb = ctx.enter_context(tc.tile_pool(name="pysb", bufs=2))
        ps_s = ctx.enter_context(tc.tile_pool(name="ps_s", bufs=2, space="PSUM"))
        ps_a = ctx.enter_context(tc.tile_pool(name="ps_a", bufs=2, space="PSUM"))
        ps_m = ctx.enter_context(tc.tile_pool(name="ps_m", bufs=2, space="PSUM"))

        msk_sb = const.tile([128, 128], BF16, tag="msk", name="msk_sb")
        nc.sync.dma_start(msk_sb[:], msk_d)

        wq_sb, wk_sb, wv_sb, bq_sb, bk_sb, wo_sb = [], [], [], [], [], []
        xt_sb, qT, kT, vaug, attnT = [], [], [], [], []
        for p in range(2):
            for lst, dram, shape, dt, nm in (
                (wq_sb, wq_d, [128, 128], BF16, "wq"),
                (wk_sb, wk_d, [128, 128], BF16, "wk"),
                (wv_sb, wv_d, [128, 128], BF16, "wv"),
                (bq_sb, bq_d, [128, 1], F32, "bq"),
                (bk_sb, bk_d, [128, 1], F32, "bk"),
                (wo_sb, wo_d, [128, D], BF16, "wo"),
            ):
                t_ = const.tile(shape, dt, tag=f"{nm}{p}", name=f"{nm}{p}_sb")
                nc.sync.dma_start(t_[:], dram[p])
                lst.append(t_)
            t_ = pers.tile([128, T], BF16, tag=f"xt{p}", name=f"xt{p}_sb")
            nc.sync.dma_start(t_[:], xt_d[128 * p : 128 * (p + 1), :])
            xt_sb.append(t_)
            qT.append(pers.tile([128, T], BF16, tag=f"qT{p}", name=f"qT{p}_sb"))
            kT.append(pers.tile([128, T], BF16, tag=f"kT{p}", name=f"kT{p}_sb"))
            vaug.append(
                pers.tile([128, 256 * TCH], BF16, tag=f"va{p}", name=f"va{p}_sb")
            )
            attnT.append(
                pers.tile([128, T], BF16, tag=f"aT{p}", name=f"aT{p}_sb")
            )

        # ---- Phase A: QKV projections (2-head block-diagonal packing) ----
        for p in range(2):
            for j in range(NQT):
                sl = slice(QTILE * j, QTILE * (j + 1))
                pq = ps_m.tile([128, QTILE], F32, tag="m", name="pq")
                nc.tensor.matmul(
                    pq[:], wq_sb[p][:], xt_sb[p][:, sl], start=True, stop=True
                )
                nc.scalar.activation(
                    qT[p][:, sl], pq[:], AF.Identity, bias=bq_sb[p][:]
                )
                pk = ps_m.tile([128, QTILE], F32, tag="m", name="pk")
                nc.tensor.matmul(
                    pk[:], wk_sb[p][:], xt_sb[p][:, sl], start=True, stop=True
                )
                nc.scalar.activation(
                    kT[p][:, sl], pk[:], AF.Identity, bias=bk_sb[p][:]
                )
            # V_aug layout per (t-chunk, head): [V_h | ones x 64] (M=128 each).
            # The ones half makes the AV^T matmul emit the softmax denominator
            # replicated across 64 partitions, rows 64:128 of its output.
            nc.gpsimd.memset(
                vaug[p].rearrange("p (g c) -> p g c", c=128)[:, :, 64:128], 1.0
            )
            for tj in range(TCH):
                pv = ps_m.tile([128, QTILE], F32, tag="m", name="pv")
                nc.tensor.matmul(
                    pv[:, 0:128],
                    xt_sb[p][:, 128 * tj : 128 * (tj + 1)],
                    wv_sb[p][:],
                    start=True,
                    stop=True,
                )
                src = pv[:, 0:128].rearrange("p (h x) -> p h x", h=2)
                dst = vaug[p][:, 256 * tj : 256 * tj + 256].rearrange(
                    "p (h c) -> p h c", h=2
                )[:, :, 0:64]
                nc.vector.tensor_copy(dst, src)

        # ---- Phase B: attention + output projection ----
        def avt(p, h, qi, kb, aps, es_t):
            # out^T = [V | 1]^T @ es accumulated over k-blocks: rows 0:64 are
            # attn@V transposed, rows 64:128 the softmax denominator
            # replicated 64x (free broadcast for the division).
            nkb = 4 * (qi + 1)
            cs = max(0, 128 * (kb - 4 * qi))
            nc.tensor.matmul(
                aps[:, cs:QTILE],
                vaug[p][:, 256 * kb + 128 * h : 256 * kb + 128 * (h + 1)],
                es_t[:, 1024 * kb + 512 * h + cs : 1024 * kb + 512 * (h + 1)],
                start=(kb == 0),
                stop=(kb == nkb - 1),
            )

        def outproj_tj(tj):
            # output projection for one 128-row t-chunk
            tsl = slice(128 * tj, 128 * (tj + 1))
            y0 = ps_m.tile([128, 512], F32, tag="m", name="y0")
            y1 = ps_m.tile([128, 512], F32, tag="m", name="y1")
            for half, yp in ((0, y0), (1, y1)):
                nsl = slice(512 * half, 512 * (half + 1))
                nc.tensor.matmul(
                    yp[:], attnT[0][:, tsl], wo_sb[0][:, nsl],
                    start=True, stop=False,
                )
                nc.tensor.matmul(
                    yp[:], attnT[1][:, tsl], wo_sb[1][:, nsl],
                    start=False, stop=True,
                )
            ysb = pysb.tile([128, D], BF16, tag="y", name="ysb")
            nc.vector.tensor_copy(ysb[:, 0:512], y0[:])
            nc.vector.tensor_copy(ysb[:, 512:1024], y1[:])
            nc.sync.dma_start(y_d[tsl, :], ysb[:])

        for qi in range(NQT):
            q0 = QTILE * qi
            nkb = 4 * (qi + 1)
            for p in range(2):
                es_t = pex.tile([128, 1024 * nkb], BF16, tag="es", name="es_t")
                aps = [
                    ps_a.tile([128, QTILE], F32, tag="a", name=f"aps{h}")
                    for h in range(2)
                ]
                # Per k-block: previous block's AV^T pair first, then the
                # row-packed scores pair back-to-back (keeps them concurrent
                # in the array), then the joint exp + causal mask strip.
                # The k-loop is exp-gated on ScalarE, so the previous q-tile's
                # output-projection chunks are sprinkled in to fill PE slack.
                for kb in range(nkb):
                    cs = max(0, 128 * (kb - 4 * qi))
                    k0 = 128 * kb
                    if p == 0 and qi > 0 and kb in (1, 3, 5, 7):
                        outproj_tj(4 * (qi - 1) + (kb - 1) // 2)
                    if kb > 0:
                        for h in range(2):
                            avt(p, h, qi, kb - 1, aps[h], es_t)
                    sps = ps_s.tile([128, 1024], F32, tag="s", name="sps")
                    for h in range(2):
                        hp = slice(64 * h, 64 * (h + 1))
                        nc.tensor.matmul(
                            sps[:, 512 * h + cs : 512 * (h + 1)],
                            kT[p][hp, k0 : k0 + 128],
                            qT[p][hp, q0 + cs : q0 + QTILE],
                            start=True,
                            stop=True,
                        )
                    nc.scalar.activation(
                        es_t[:, 1024 * kb : 1024 * (kb + 1)].rearrange(
                            "p (h x) -> p h x", h=2
                        )[:, :, cs:512],
                        sps.rearrange("p (h x) -> p h x", h=2)[:, :, cs:512],
                        AF.Exp,
                    )
                    if kb >= 4 * qi:  # diagonal block: mask the 128-strips
                        for h in range(2):
                            stp = slice(
                                1024 * kb + 512 * h + cs,
                                1024 * kb + 512 * h + cs + 128,
                            )
                            nc.gpsimd.tensor_mul(
                                es_t[:, stp], es_t[:, stp], msk_sb[:]
                            )
                for h in range(2):
                    avt(p, h, qi, nkb - 1, aps[h], es_t)
                for h in range(2):
                    # custom-DVE recip ucode ignores base_partition 64 on HW,
                    # so shift the sums to base 0 with a plain copy first
                    sms = patt.tile([64, QTILE], F32, tag="sms", name="sms")
                    rec = patt.tile([64, QTILE], F32, tag="rec", name="rec")
                    nc.vector.tensor_copy(sms[:], aps[h][64:128, :])
                    nc.vector.reciprocal_approx_fast(rec[:], sms[:])
                    nc.vector.tensor_mul(
                        attnT[p][64 * h : 64 * (h + 1), q0 : q0 + QTILE],
                        aps[h][0:64, :],
                        rec[:],
                    )
        for tj in range(4 * (NQT - 1), 4 * NQT):
            outproj_tj(tj)

    nc.compile()
    return nc


def _host_prep(x, Wq, bq, Wk, bk, Wv, bv, Wo, bo):
    x = np.asarray(x, np.float32)
    Wq, bq = np.asarray(Wq, np.float32), np.asarray(bq, np.float32)
    Wk, bk = np.asarray(Wk, np.float32), np.asarray(bk, np.float32)
    Wv, bv = np.asarray(Wv, np.float32), np.asarray(bv, np.float32)
    Wo, bo = np.asarray(Wo, np.float32), np.asarray(bo, np.float32)
    msk = np.triu(np.ones((128, 128), np.float32)).astype(bf16)
    in_maps = []
    for c in range(NCORES):
        b, g = divmod(c, 4)
        h0 = 4 * g
        xt = np.ascontiguousarray(x[b, :, 256 * g : 256 * (g + 1)].T).astype(bf16)
        wqs = np.zeros((2, 128, 128), np.float32)
        wks = np.zeros((2, 128, 128), np.float32)
        wvs = np.zeros((2, 128, 128), np.float32)
        bqs = np.zeros((2, 128, 1), np.float32)
        bks = np.zeros((2, 128, 1), np.float32)
        for p in range(2):
            ha, hb = h0 + 2 * p, h0 + 2 * p + 1
            wqs[p, 0:64, 0:64] = Wq[ha] * 0.125
            wqs[p, 64:128, 64:128] = Wq[hb] * 0.125
            wks[p, 0:64, 0:64] = Wk[ha]
            wks[p, 64:128, 64:128] = Wk[hb]
            wvs[p, 0:64, 0:64] = Wv[ha]
            wvs[p, 64:128, 64:128] = Wv[hb]
            bqs[p, 0:64, 0] = bq[ha] * 0.125
            bqs[p, 64:128, 0] = bq[hb] * 0.125
            bks[p, 0:64, 0] = bk[ha]
            bks[p, 64:128, 0] = bk[hb]
        wo_c = np.ascontiguousarray(
            Wo[256 * g : 256 * (g + 1)].reshape(2, 128, D)
        ).astype(bf16)
        in_maps.append(
            {
                "xt": xt,
                "wq": wqs.astype(bf16),
                "wk": wks.astype(bf16),
                "wv": wvs.astype(bf16),
                "bq": bqs,
                "bk": bks,
                "wo": wo_c,
                "msk": msk,
            }
        )
    # bv contributes bv_flat @ Wo to every output row (softmax weights sum to 1)
    bo_eff = bo + bv.reshape(-1) @ Wo
    return in_maps, bo_eff


def _finalize(results, bo_eff):
    out = np.zeros((B, T, D), np.float32)
    for c in range(NCORES):
        out[c // 4] += np.asarray(results[c]["y"], dtype=np.float32)
    out += bo_eff[None, None, :]
    return out


def kernel(**inputs):
    if "nc" not in _CACHE:
        _CACHE["nc"] = _build()
    nc = _CACHE["nc"]
    in_maps, bo_eff = _host_prep(**inputs)
    res = run_bass_kernel_spmd(
        nc, in_maps, core_ids=list(range(NCORES)), trace=False
    )
    return _finalize(res.results, bo_eff)


def kernel_traced(**inputs):
    """Dev helper: run with NTFF profiling, return (out, exec_time_ns, tmpdir)."""
    import glob
    import tempfile

    from concourse import bass2jax
    from trn_agent_boot.trn_boot import _ntff_profile_via_ctypes

    if "nc" not in _CACHE:
        _CACHE["nc"] = _build()
    nc = _CACHE["nc"]
    in_maps, bo_eff = _host_prep(**inputs)
    hook = _ntff_profile_via_ctypes("/opt/axon/libaxon_pjrt.so")
    tmpdir = tempfile.mkdtemp(prefix="mha_trace_")
    with hook(tmpdir, [0]):
        results = bass2jax.run_bass_via_pjrt(nc, in_maps, n_cores=NCORES)
    out = _finalize(results, bo_eff)

    exec_time_ns = None
    try:
        import gauge.profiler
        from concourse._compat import FishPath

        ntffs = glob.glob(f"{tmpdir}/*.ntff")
        if ntffs:
            profile = gauge.profiler.Profile(
                profile_path=FishPath(tmpdir),
                kernel_dev_mode=True,
                profile_on_exit=False,
                bass_kernel=nc.m,
                offline_processing=True,
                fname="*_body*",
            )
            pres = profile.to_perfetto(model_index=(0,))
            if pres:
                exec_time_ns = pres[0].exec_time_ns
    except Exception as e:  # profiling is best-effort
        print(f"profile processing failed: {type(e).__name__}: {e}")
    return out, exec_time_ns, tmpdir


# revision 27
# speedup vs baseline: 1.0958x; 1.0958x over previous
"""Trainium2 Bass kernel for causal multi-head attention (B=2, T=2048, D=1024, H=16).

Sharding: 8 cores = 2 batches x 4 head-groups. Each core computes 4 heads
(as 2 head-pairs packed into 128 partitions) of one batch, plus its row-shard
of the output projection; the host sums the 4 partial outputs per batch.

Compute is bf16 with fp32 PSUM accumulation. Softmax uses no max-subtraction
(scores ~ N(0,1), exp is safe in fp32) and gets the denominator for free via
an all-ones column appended to V.
"""

import numpy as np
import ml_dtypes
from contextlib import ExitStack

import concourse.bass as bass
import concourse.mybir as mybir
import concourse.tile as tile
from concourse import bacc
from concourse.bass_utils import run_bass_kernel_spmd

BF16 = mybir.dt.bfloat16
F32 = mybir.dt.float32
AF = mybir.ActivationFunctionType
bf16 = ml_dtypes.bfloat16

B, T, D, H, DH = 2, 2048, 1024, 16, 64
NCORES = 8
QTILE = 512          # q columns per score tile
NQT = T // QTILE     # 4
TCH = T // 128       # 16 t-chunks / k-blocks

_CACHE = {}


def _build():
    nc = bacc.Bacc(
        "TRN2", target_bir_lowering=False, debug=False, num_devices=NCORES
    )
    xt_d = nc.dram_tensor("xt", [256, T], BF16, kind="ExternalInput").ap()
    wq_d = nc.dram_tensor("wq", [2, 128, 128], BF16, kind="ExternalInput").ap()
    wk_d = nc.dram_tensor("wk", [2, 128, 128], BF16, kind="ExternalInput").ap()
    wv_d = nc.dram_tensor("wv", [2, 128, 128], BF16, kind="ExternalInput").ap()
    bq_d = nc.dram_tensor("bq", [2, 128, 1], F32, kind="ExternalInput").ap()
    bk_d = nc.dram_tensor("bk", [2, 128, 1], F32, kind="ExternalInput").ap()
    wo_d = nc.dram_tensor("wo", [2, 128, D], BF16, kind="ExternalInput").ap()
    msk_d = nc.dram_tensor("msk", [128, 128], BF16, kind="ExternalInput").ap()
    y_d = nc.dram_tensor("y", [T, D], BF16, kind="ExternalOutput").ap()

    with tile.TileContext(nc) as tc, ExitStack() as ctx:
        const = ctx.enter_context(tc.tile_pool(name="const", bufs=1))
        pers = ctx.enter_context(tc.tile_pool(name="pers", bufs=1))
        pex = ctx.enter_context(tc.tile_pool(name="pex", bufs=2))
        patt = ctx.enter_context(tc.tile_pool(name="patt", bufs=4))
        pysb = ctx.enter_context(tc.tile_pool(name="pysb", bufs=2))
        ps_s = ctx.enter_context(tc.tile_pool(name="ps_s", bufs=2, space="PSUM"))
        ps_a = ctx.enter_context(tc.tile_pool(name="ps_a", bufs=2, space="PSUM"))
        ps_m = ctx.enter_context(tc.tile_pool(name="ps_m", bufs=2, space="PSUM"))

        msk_sb = const.tile([128, 128], BF16, tag="msk", name="msk_sb")
        nc.sync.dma_start(msk_sb[:], msk_d)

        wq_sb, wk_sb, wv_sb, bq_sb, bk_sb, wo_sb = [], [], [], [], [], []
        xt_sb, qT, kT, vaug, attnT = [], [], [], [], []
        for p in range(2):
            for lst, dram, shape, dt, nm in (
                (wq_sb, wq_d, [128, 128], BF16, "wq"),
                (wk_sb, wk_d, [128, 128], BF16, "wk"),
                (wv_sb, wv_d, [128, 128], BF16, "wv"),
                (bq_sb, bq_d, [128, 1], F32, "bq"),
                (bk_sb, bk_d, [128, 1], F32, "bk"),
                (wo_sb, wo_d, [128, D], BF16, "wo"),
            ):
                t_ = const.tile(shape, dt, tag=f"{nm}{p}", name=f"{nm}{p}_sb")
                nc.sync.dma_start(t_[:], dram[p])
                lst.append(t_)
            t_ = pers.tile([128, T], BF16, tag=f"xt{p}", name=f"xt{p}_sb")
            for j in range(4):  # split so loads spread across DMA queues
                sl = slice(QTILE * j, QTILE * (j + 1))
                nc.sync.dma_start(t_[:, sl], xt_d[128 * p : 128 * (p + 1), sl])
            xt_sb.append(t_)
            qT.append(pers.tile([128, T], BF16, tag=f"qT{p}", name=f"qT{p}_sb"))
            kT.append(pers.tile([128, T], BF16, tag=f"kT{p}", name=f"kT{p}_sb"))
            vaug.append(
                pers.tile([128, 256 * TCH], BF16, tag=f"va{p}", name=f"va{p}_sb")
            )
            attnT.append(
                pers.tile([128, T], BF16, tag=f"aT{p}", name=f"aT{p}_sb")
            )

        # ---- Phase A: QKV projections (2-head block-diagonal packing) ----
        for p in range(2):
            for j in range(NQT):
                sl = slice(QTILE * j, QTILE * (j + 1))
                pq = ps_m.tile([128, QTILE], F32, tag="m", name="pq")
                nc.tensor.matmul(
                    pq[:], wq_sb[p][:], xt_sb[p][:, sl], start=True, stop=True
                )
                nc.scalar.activation(
                    qT[p][:, sl], pq[:], AF.Identity, bias=bq_sb[p][:]
                )
                pk = ps_m.tile([128, QTILE], F32, tag="m", name="pk")
                nc.tensor.matmul(
                    pk[:], wk_sb[p][:], xt_sb[p][:, sl], start=True, stop=True
                )
                nc.scalar.activation(
                    kT[p][:, sl], pk[:], AF.Identity, bias=bk_sb[p][:]
                )
            # V_aug layout per (t-chunk, head): [V_h | ones x 64] (M=128 each).
            # The ones half makes the AV^T matmul emit the softmax denominator
            # replicated across 64 partitions, rows 64:128 of its output.
            nc.gpsimd.memset(
                vaug[p].rearrange("p (g c) -> p g c", c=128)[:, :, 64:128], 1.0
            )
            for tj in range(TCH):
                pv = ps_m.tile([128, QTILE], F32, tag="m", name="pv")
                nc.tensor.matmul(
                    pv[:, 0:128],
                    xt_sb[p][:, 128 * tj : 128 * (tj + 1)],
                    wv_sb[p][:],
                    start=True,
                    stop=True,
                )
                src = pv[:, 0:128].rearrange("p (h x) -> p h x", h=2)
                dst = vaug[p][:, 256 * tj : 256 * tj + 256].rearrange(
                    "p (h c) -> p h c", h=2
                )[:, :, 0:64]
                nc.vector.tensor_copy(dst, src)

        # ---- Phase B: attention + output projection ----
        def avt(p, h, qi, kb, aps, es_t):
            # out^T = [V | 1]^T @ es accumulated over k-blocks: rows 0:64 are
            # attn@V transposed, rows 64:128 the softmax denominator
            # replicated 64x (free broadcast for the division).
            nkb = 4 * (qi + 1)
            cs = max(0, 128 * (kb - 4 * qi))
            nc.tensor.matmul(
                aps[:, cs:QTILE],
                vaug[p][:, 256 * kb + 128 * h : 256 * kb + 128 * (h + 1)],
                es_t[:, 1024 * kb + 512 * h + cs : 1024 * kb + 512 * (h + 1)],
                start=(kb == 0),
                stop=(kb == nkb - 1),
            )

        def outproj_tj(tj):
            # output projection for one 128-row t-chunk
            tsl = slice(128 * tj, 128 * (tj + 1))
            y0 = ps_m.tile([128, 512], F32, tag="m", name="y0")
            y1 = ps_m.tile([128, 512], F32, tag="m", name="y1")
            for half, yp in ((0, y0), (1, y1)):
                nsl = slice(512 * half, 512 * (half + 1))
                nc.tensor.matmul(
                    yp[:], attnT[0][:, tsl], wo_sb[0][:, nsl],
                    start=True, stop=False,
                )
                nc.tensor.matmul(
                    yp[:], attnT[1][:, tsl], wo_sb[1][:, nsl],
                    start=False, stop=True,
                )
            ysb = pysb.tile([128, D], BF16, tag="y", name="ysb")
            nc.vector.tensor_copy(ysb[:, 0:512], y0[:])
            nc.vector.tensor_copy(ysb[:, 512:1024], y1[:])
            nc.sync.dma_start(y_d[tsl, :], ysb[:])

        for qi in range(NQT):
            q0 = QTILE * qi
            nkb = 4 * (qi + 1)
            for p in range(2):
                es_t = pex.tile([128, 1024 * nkb], BF16, tag="es", name="es_t")
                aps = [
                    ps_a.tile([128, QTILE], F32, tag="a", name=f"aps{h}")
                    for h in range(2)
                ]
                # Per k-block: previous block's AV^T pair first, then the
                # row-packed scores pair back-to-back (keeps them concurrent
                # in the array), then the joint exp + causal mask strip.
                # The k-loop is exp-gated on ScalarE, so the previous q-tile's
                # output-projection chunks are sprinkled in to fill PE slack.
                for kb in range(nkb):
                    cs = max(0, 128 * (kb - 4 * qi))
                    k0 = 128 * kb
                    if kb > 0:
                        for h in range(2):
                            avt(p, h, qi, kb - 1, aps[h], es_t)
                    sps = ps_s.tile([128, 1024], F32, tag="s", name="sps")
                    for h in range(2):
                        hp = slice(64 * h, 64 * (h + 1))
                        nc.tensor.matmul(
                            sps[:, 512 * h + cs : 512 * (h + 1)],
                            kT[p][hp, k0 : k0 + 128],
                            qT[p][hp, q0 + cs : q0 + QTILE],
                            start=True,
                            stop=True,
                        )
                    nc.scalar.activation(
                        es_t[:, 1024 * kb : 1024 * (kb + 1)].rearrange(
                            "p (h x) -> p h x", h=2
                        )[:, :, cs:512],
                        sps.rearrange("p (h x) -> p h x", h=2)[:, :, cs:512],
                        AF.Exp,
                    )
                    if kb >= 4 * qi:  # diagonal block: mask the 128-strips
                        for h in range(2):
                            stp = slice(
                                1024 * kb + 512 * h + cs,
                                1024 * kb + 512 * h + cs + 128,
                            )
                            nc.gpsimd.tensor_mul(
                                es_t[:, stp], es_t[:, stp], msk_sb[:]
                            )
                if p == 0 and qi > 0:
                    for tj in range(4 * (qi - 1), 4 * qi):
                        outproj_tj(tj)  # deferred: divisions are long done
                for h in range(2):
                    avt(p, h, qi, nkb - 1, aps[h], es_t)
                for h in range(2):
                    # custom-DVE recip ucode ignores base_partition 64 on HW,
                    # so shift the sums to base 0 with a plain copy first
                    sms = patt.tile([64, QTILE], F32, tag="sms", name="sms")
                    rec = patt.tile([64, QTILE], F32, tag="rec", name="rec")
                    nc.vector.tensor_copy(sms[:], aps[h][64:128, :])
                    nc.vector.reciprocal_approx_fast(rec[:], sms[:])
                    nc.vector.tensor_mul(
                        attnT[p][64 * h : 64 * (h + 1), q0 : q0 + QTILE],
                        aps[h][0:64, :],
                        rec[:],
                    )
        for tj in range(4 * (NQT - 1), 4 * NQT):
            outproj_tj(tj)

    nc.compile()
    return nc


def _host_prep(x, Wq, bq, Wk, bk, Wv, bv, Wo, bo):
    x = np.asarray(x, np.float32)
    Wq, bq = np.asarray(Wq, np.float32), np.asarray(bq, np.float32)
    Wk, bk = np.asarray(Wk, np.float32), np.asarray(bk, np.float32)
    Wv, bv = np.asarray(Wv, np.float32), np.asarray(bv, np.float32)
    Wo, bo = np.asarray(Wo, np.float32), np.asarray(bo, np.float32)
    msk = np.triu(np.ones((128, 128), np.float32)).astype(bf16)
    in_maps = []
    for c in range(NCORES):
        b, g = divmod(c, 4)
        h0 = 4 * g
        xt = np.ascontiguousarray(x[b, :, 256 * g : 256 * (g + 1)].T).astype(bf16)
        wqs = np.zeros((2, 128, 128), np.float32)
        wks = np.zeros((2, 128, 128), np.float32)
        wvs = np.zeros((2, 128, 128), np.float32)
        bqs = np.zeros((2, 128, 1), np.float32)
        bks = np.zeros((2, 128, 1), np.float32)
        for p in range(2):
            ha, hb = h0 + 2 * p, h0 + 2 * p + 1
            wqs[p, 0:64, 0:64] = Wq[ha] * 0.125
            wqs[p, 64:128, 64:128] = Wq[hb] * 0.125
            wks[p, 0:64, 0:64] = Wk[ha]
            wks[p, 64:128, 64:128] = Wk[hb]
            wvs[p, 0:64, 0:64] = Wv[ha]
            wvs[p, 64:128, 64:128] = Wv[hb]
            bqs[p, 0:64, 0] = bq[ha] * 0.125
            bqs[p, 64:128, 0] = bq[hb] * 0.125
            bks[p, 0:64, 0] = bk[ha]
            bks[p, 64:128, 0] = bk[hb]
        wo_c = np.ascontiguousarray(
            Wo[256 * g : 256 * (g + 1)].reshape(2, 128, D)
        ).astype(bf16)
        in_maps.append(
            {
                "xt": xt,
                "wq": wqs.astype(bf16),
                "wk": wks.astype(bf16),
                "wv": wvs.astype(bf16),
                "bq": bqs,
                "bk": bks,
                "wo": wo_c,
                "msk": msk,
            }
        )
    # bv contributes bv_flat @ Wo to every output row (softmax weights sum to 1)
    bo_eff = bo + bv.reshape(-1) @ Wo
    return in_maps, bo_eff


def _finalize(results, bo_eff):
    out = np.zeros((B, T, D), np.float32)
    for c in range(NCORES):
        out[c // 4] += np.asarray(results[c]["y"], dtype=np.float32)
    out += bo_eff[None, None, :]
    return out


def kernel(**inputs):
    if "nc" not in _CACHE:
        _CACHE["nc"] = _build()
    nc = _CACHE["nc"]
    in_maps, bo_eff = _host_prep(**inputs)
    res = run_bass_kernel_spmd(
        nc, in_maps, core_ids=list(range(NCORES)), trace=False
    )
    return _finalize(res.results, bo_eff)


def kernel_traced(**inputs):
    """Dev helper: run with NTFF profiling, return (out, exec_time_ns, tmpdir)."""
    import glob
    import tempfile

    from concourse import bass2jax
    from trn_agent_boot.trn_boot import _ntff_profile_via_ctypes

    if "nc" not in _CACHE:
        _CACHE["nc"] = _build()
    nc = _CACHE["nc"]
    in_maps, bo_eff = _host_prep(**inputs)
    hook = _ntff_profile_via_ctypes("/opt/axon/libaxon_pjrt.so")
    tmpdir = tempfile.mkdtemp(prefix="mha_trace_")
    with hook(tmpdir, [0]):
        results = bass2jax.run_bass_via_pjrt(nc, in_maps, n_cores=NCORES)
    out = _finalize(results, bo_eff)

    exec_time_ns = None
    try:
        import gauge.profiler
        from concourse._compat import FishPath

        ntffs = glob.glob(f"{tmpdir}/*.ntff")
        if ntffs:
            profile = gauge.profiler.Profile(
                profile_path=FishPath(tmpdir),
                kernel_dev_mode=True,
                profile_on_exit=False,
                bass_kernel=nc.m,
                offline_processing=True,
                fname="*_body*",
            )
            pres = profile.to_perfetto(model_index=(0,))
            if pres:
                exec_time_ns = pres[0].exec_time_ns
    except Exception as e:  # profiling is best-effort
        print(f"profile processing failed: {type(e).__name__}: {e}")
    return out, exec_time_ns, tmpdir
